# revision 14
# baseline (speedup 1.0000x reference)
"""Trainium2 Bass kernel for nn_HOR_16870631539538 (dense_transformer).

Module (per batch item b, C=64 channels, hw=4096 spatial):
  stage 1: p = x_low^T conv outputs attention [hw,hw], softmax over axis n,
           e = p_sm @ v + x_low
  stage 2: t = conv_e(e) @ xl2_sp  (64x64), softmax over c, out = x_mid @ t_sm

Sharding: 8 cores = 4 batch items x 2 halves of the softmax-column dim (m).
Host permutes spatial columns per core so its m-half is always cols [0, MH);
everything downstream (n order, G contraction, output half) is permutation-
consistent. Downstream needs only G = e @ xl2_sp (64x65 incl. ones-row for the
e_conv bias), linear in the m-partial e, so the cross-core combine is ONE 16KB
AllReduce of G.

Perf notes (vs. the fp32r baseline, 260us):
 - attention matmuls in fp16 (fp32r streams ~3x slower on HW than 16-bit)
 - conv bias folded into the matmul via a ones-row on the inputs (K=65)
 - exp WITHOUT accumulator read; softmax denominators via DVE reduce on the
   bf16 slab (2 elem/cycle)
 - x_low residual folded into the psum accumulators via a 0.5*I fp16 matmul
 - all transposes (v, xl2, e) done by XBAR DMA-transpose, not the PE
 - input DMA chunked so convs overlap the loads; conv evictions split DVE/ACT
 - stage-2 computes t pre-transposed (softmax axis on the free dim directly)
"""

import numpy as np

import concourse.bacc as bacc
import concourse.bass as bass
import concourse.mybir as mybir
import concourse.tile as tile
from concourse.bass_utils import run_bass_kernel_spmd

dt = mybir.dt
AF = mybir.ActivationFunctionType
ALU = mybir.AluOpType

N_CORES = 8
C = 64
HW = 4096
MH = HW // 2           # per-core m-half (2048)
NCHUNK = MH // 128     # 16 m-chunks of 128 rows

f32 = dt.float32
f32r = dt.float32r
f16 = dt.float16
bf16 = dt.bfloat16

USE_COLLECTIVE = True

_CACHE = {}


def build():
    nc = bacc.Bacc("TRN2", target_bir_lowering=False, debug=False,
                   num_devices=N_CORES)

    def din(name, shape, dtype=f32):
        return nc.dram_tensor(name, shape, dtype, kind="ExternalInput").ap()

    io = {}
    io["xin"] = din("xin", [C + 1, HW], f32r)    # x[b] permuted + ones row
    io["xlat"] = din("xlat", [C + 1, HW], f32r)  # x_latter[b] permuted + ones
    io["wb_all"] = din("wb_all", [C + 1, 5 * C], f32r)  # [W.T; b] x5 convs
    io["weT"] = din("weT", [C, C])
    io["be_t"] = din("be_t", [1, C])             # e_conv bias row
    io["idf32"] = din("idf32", [C, C])
    io["outp"] = nc.dram_tensor("outp", [C, MH], f32,
                                kind="ExternalOutput").ap()

    with tile.TileContext(nc) as tc:
        _body(nc, tc, io)
    nc.compile()
    return nc


def _body(nc, tc, io):
    ts = bass.ts

    const = tc.alloc_tile_pool(name="const", bufs=1)
    big = tc.alloc_tile_pool(name="big", bufs=1)
    slabp = tc.alloc_tile_pool(name="slabp", bufs=2)
    mm = tc.alloc_tile_pool(name="mm", bufs=2, space="PSUM")
    acc = tc.alloc_tile_pool(name="acc", bufs=1, space="PSUM")
    dram = tc.alloc_tile_pool(name="dram", bufs=1, space="DRAM")

    # ---- constants (gpsimd SWDGE DMAs race engine writers/readers at close
    # range; keep every DMA on the SP/ACT hwdge queues where deps work) ----
    def cload(name, shape, dtype=f32):
        t = const.tile(shape, dtype, tag=name)
        nc.sync.dma_start(t[:], io[name])
        return t

    wb_all = cload("wb_all", [C + 1, 5 * C], f32r)
    wbl = wb_all[:, 0:C]
    wbh = wb_all[:, C:2 * C]
    wbv = wb_all[:, 2 * C:3 * C]
    wblat = wb_all[:, 3 * C:4 * C]
    wbm = wb_all[:, 4 * C:5 * C]

    # ---- inputs: chunked [65, 1024] DMAs so convs start early (loop-gating
    # chunks first, remaining consts after) ----
    xin = big.tile([C + 1, HW], f32r, tag="xin")
    xlat = big.tile([C + 1, HW], f32r, tag="xlat")
    for j in (0, 1):
        nc.sync.dma_start(xin[:, ts(j, 1024)], io["xin"][:, ts(j, 1024)])
    for j in (0, 1):
        nc.sync.dma_start(xlat[:, ts(j, 1024)], io["xlat"][:, ts(j, 1024)])
    for j in (2, 3):
        nc.sync.dma_start(xin[:, ts(j, 1024)], io["xin"][:, ts(j, 1024)])
    weT = cload("weT", [C, C])
    idf32 = cload("idf32", [C, C])
    be_t = const.tile([C + 1, C], f32, tag="be_t")
    nc.sync.dma_start(be_t[C:C + 1, :], io["be_t"])
    for j in (2, 3):
        nc.sync.dma_start(xlat[:, ts(j, 1024)], io["xlat"][:, ts(j, 1024)])

    # ---- SBUF tiles ----
    xlowT16 = big.tile([C, HW], f16, tag="xlowT16")
    xl_hi16 = big.tile([C, MH], f16, tag="xl_hi16")
    v_s = big.tile([C, MH], f32, tag="v_s")
    xl2_16 = big.tile([C, HW], f16, tag="xl2_16")
    xmidT16 = big.tile([C, MH], bf16, tag="xmidT16")
    sacc = big.tile([C, 4], f32, tag="sacc")
    xlow_acc = big.tile([128, 4 * 512], f16, tag="xlow_acc")
    v_sp = big.tile([128, NCHUNK, C], f16, tag="v_sp")
    xl2sp = big.tile([128, 32, C], f16, tag="xl2sp")
    e_h = big.tile([128, 4 * 512], f32, tag="e_h")
    e_h2 = big.tile([C, 4 * 512], f32, tag="e_h2")
    e_sp = big.tile([128, 32, C], f16, tag="e_sp")

    # ---- conv: psum[c,1024] = wb^T @ x-chunk (bias via ones row), evict ----
    def conv_pass(dst, wb, src, j, evict, accum=None):
        pt = mm.tile([C, 1024], f32, tag="mmt")
        for k in range(2):
            nc.tensor.matmul(pt[:, ts(k, 512)], wb,
                             src[:, j * 1024 + k * 512:j * 1024 + (k + 1) * 512],
                             start=True, stop=True)
        if evict == "dve":
            nc.vector.tensor_copy(dst[:, ts(j, 1024)], pt[:])
        else:
            kw = {} if accum is None else {"accum_out": accum[:, j:j + 1]}
            nc.scalar.activation(dst[:, ts(j, 1024)], pt[:], AF.Copy, **kw)

    # order chosen so the loop-gating tensors (xlowT, xl_hi, v, v_sp) are
    # ready first; xl2 (tail-only) fills the remaining head time
    conv_pass(xlowT16, wbl, xin, 0, "dve")
    conv_pass(xlowT16, wbl, xin, 1, "dve")
    conv_pass(xl_hi16, wbh, xlat, 0, "act")
    conv_pass(v_s, wbv, xin, 0, "dve")
    conv_pass(xlowT16, wbl, xin, 2, "dve")
    conv_pass(xlowT16, wbl, xin, 3, "dve")
    conv_pass(xl_hi16, wbh, xlat, 1, "act")
    conv_pass(v_s, wbv, xin, 1, "dve")
    for j in range(4):
        conv_pass(xl2_16, wblat, xlat, j, "act", accum=sacc)

    # v transposed on the PE (its readers start immediately with the loop;
    # XBAR DMA-transpose completion does not gate cross-engine readers, so it
    # is only safe when the reader runs much later -- like xl2sp below)
    for g in range(0, NCHUNK, 8):
        pt = mm.tile([128, 512], f32, tag="mmt")
        for q in range(8):
            nc.tensor.transpose(pt[:, ts(q, C)], v_s[:, ts(g + q, 128)],
                                idf32[:])
        nc.vector.tensor_copy(v_sp[:, g:g + 8, :], pt[:])
    for q in range(2):
        nc.sync.dma_start(xl2sp[:, q * 16:(q + 1) * 16, :],
                          xl2_16[:, ts(q, 2048)], transpose=True)

    # x_low residual in acc layout (partition halves = even/odd n-blocks)
    for k in range(4):
        nc.sync.dma_start(xlow_acc[0:64, ts(k, 512)],
                          xlowT16[:, ts(2 * k, 512)])
        nc.sync.dma_start(xlow_acc[64:128, ts(k, 512)],
                          xlowT16[:, ts(2 * k + 1, 512)])

    # ---- stage-1 m-loop ----
    accs = [acc.tile([128, 512], f32, tag=f"acc{k}", name=f"acc{k}")
            for k in range(4)]

    def emit_eacc(slab, vs, last, first=False):
        for k in range(4):
            for p in range(2):
                blk = 2 * k + p
                nc.tensor.matmul(accs[k][p * 64:(p + 1) * 64, :], vs[:],
                                 slab[:, ts(blk, 512)], start=first,
                                 stop=last, skip_group_check=True)

    prev = None
    for i in range(NCHUNK):
        slab = slabp.tile([128, HW], bf16, tag="slab")
        dacc = slabp.tile([128, 4], f32, tag="dacc")
        for j in range(4):
            pt = mm.tile([128, 1024], f32, tag="mmt")
            for k in range(2):
                nb = 2 * j + k
                nc.tensor.matmul(pt[:, ts(k, 512)], xl_hi16[:, ts(i, 128)],
                                 xlowT16[:, ts(nb, 512)],
                                 start=True, stop=True)
            nc.scalar.activation(slab[:, ts(j, 1024)], pt[:], AF.Exp,
                                 accum_out=dacc[:, j:j + 1])
        dsum = slabp.tile([128, 1], f32, tag="dsum")
        nc.vector.reduce_sum(dsum[:], dacc[:], axis=mybir.AxisListType.X)
        rec = slabp.tile([128, 1], f32, tag="rec")
        nc.vector.reciprocal(rec[:], dsum[:])
        vs = slabp.tile([128, C], bf16, tag="vs")
        nc.vector.tensor_scalar(vs[:], v_sp[:, i, :], rec[:], None, ALU.mult)
        if prev is not None:
            emit_eacc(*prev, first=(i == 1))
        prev = (slab, vs, i == NCHUNK - 1)
    emit_eacc(*prev)

    # ---- tail: evict e (+ 0.5*x_low residual), PE transposes, G ----
    for k in range(4):
        nc.vector.scalar_tensor_tensor(e_h[:, ts(k, 512)],
                                       xlow_acc[:, ts(k, 512)], 0.5,
                                       accs[k][:], ALU.mult, ALU.add)
    # odd n-blocks (partitions 64-127) down to 0-63 via SBUF DMA
    nc.sync.dma_start(e_h2[:], e_h[64:128, :])

    # s row (xl2 row-sums for the e_conv bias term), staged pre-collective
    s_col = big.tile([C, 1], f32, tag="s_col")
    nc.vector.reduce_sum(s_col[:], sacc[:], axis=mybir.AxisListType.X)
    spt_ps = mm.tile([128, 512], f32, tag="mmt")
    nc.tensor.transpose(spt_ps[0:1, 0:C], s_col[:], idf32[:])
    gs_stage = big.tile([C + 1, C], f32, tag="gs_stage")
    nc.vector.tensor_scalar(gs_stage[C:C + 1, :], spt_ps[0:1, 0:C], 0.5,
                            None, ALU.mult)

    for g in range(0, 32, 8):
        pt = mm.tile([128, 512], f32, tag="mmt")
        for q in range(8):
            t_idx = g + q
            blk, sl = t_idx // 4, t_idx % 4
            kk, p = blk // 2, blk % 2
            src = e_h if p == 0 else e_h2
            nc.tensor.transpose(
                pt[:, ts(q, C)],
                src[0:C, kk * 512 + sl * 128:kk * 512 + (sl + 1) * 128],
                idf32[:])
        nc.vector.tensor_copy(e_sp[:, g:g + 8, :], pt[:])
    gps = acc.tile([128, 512], f32, tag="acc0", name="acc0g")
    G = gps[0:64, 0:64]
    for t_idx in range(32):
        nc.tensor.matmul(G, e_sp[:, t_idx, :], xl2sp[:, t_idx, :],
                         start=(t_idx == 0), stop=(t_idx == 31),
                         skip_group_check=True)
    nc.vector.tensor_copy(gs_stage[0:C, :], G)

    # ---- AllReduce G over core pairs ----
    gs_red = big.tile([C + 1, C], f32, tag="gs_red")
    if USE_COLLECTIVE:
        g_in = dram.tile([C + 1, C], f32, tag="g_in")
        g_out = dram.tile([C + 1, C], f32, tag="g_out")
        nc.sync.dma_start(g_in[:], gs_stage[:])
        nc.gpsimd.collective_compute(
            "AllReduce", ALU.add,
            ins=[g_in.opt()], outs=[g_out.opt()],
            replica_groups=[[0, 1], [2, 3], [4, 5], [6, 7]],
        )
        nc.sync.dma_start(gs_red[:], g_out[:])
    else:
        nc.vector.tensor_copy(gs_red[:], gs_stage[:])

    # xmid conv overlaps the collective
    for j in range(2):
        conv_pass(xmidT16, wbm, xin, j, "act")

    # ---- tT[d,c] = (We @ G + be x s)^T, computed directly transposed ----
    tps = mm.tile([128, 512], f32, tag="mmt")
    tT = tps[0:C, 0:C]
    nc.tensor.matmul(tT, gs_red[0:C, :], weT[:], start=True, stop=False,
                     skip_group_check=True)
    nc.tensor.matmul(tT, gs_red[C:C + 1, :], be_t[C:C + 1, :], start=False,
                     stop=True, tile_position=(64, 0), skip_group_check=True)

    # softmax over c (free dim)
    nmax = big.tile([C, 1], f32, tag="nmax")
    nc.vector.reduce_max(nmax[:], tT, axis=mybir.AxisListType.X, negate=True)
    texp = big.tile([C, C], f32, tag="texp")
    tsum = big.tile([C, 1], f32, tag="tsum")
    nc.scalar.activation(texp[:], tT, AF.Exp, bias=nmax[:], accum_out=tsum[:])
    trec = big.tile([C, 1], f32, tag="trec")
    nc.vector.reciprocal(trec[:], tsum[:])
    tsmT = big.tile([C, C], f32, tag="tsmT")
    nc.vector.tensor_scalar(tsmT[:], texp[:], trec[:], None, ALU.mult)
    tb = mm.tile([128, 512], f32, tag="mmt")
    nc.tensor.transpose(tb[0:C, 0:C], tsmT[:], idf32[:])
    tsm16 = big.tile([C, C], bf16, tag="tsm16")
    nc.vector.tensor_copy(tsm16[:], tb[0:C, 0:C])

    # ---- out^T[d, n-half] = tsm^T @ xmidT; per-block copy + DMA out ----
    osb = big.tile([C, MH], f32, tag="osb")
    for k in range(4):
        op = mm.tile([C, 512], f32, tag="mmt")
        nc.tensor.matmul(op[:], tsm16[:], xmidT16[:, ts(k, 512)],
                         start=True, stop=True)
        if k % 2 == 0:
            nc.vector.tensor_copy(osb[:, ts(k, 512)], op[:])
        else:
            nc.scalar.activation(osb[:, ts(k, 512)], op[:], AF.Copy)
        nc.sync.dma_start(io["outp"][:, ts(k, 512)], osb[:, ts(k, 512)])

    for p in (dram, acc, mm, slabp, big, const):
        p.release()


def _prep_inputs(x_latter, x, W, b):
    """Build the 8 per-core input maps from full inputs."""
    B = x_latter.shape[0]
    xr = np.ascontiguousarray(x.reshape(B, C, HW))
    xlr = np.ascontiguousarray(x_latter.reshape(B, C, HW))
    worder = ["low", "high", "value", "latter", "mid"]
    wb_all = np.ascontiguousarray(np.hstack(
        [np.vstack([W[n].T, b[n].reshape(1, C)]) for n in worder]
    ).astype(np.float32))
    weT = np.ascontiguousarray(W["e_conv"].T)
    be_t = np.ascontiguousarray(b["e_conv"].reshape(1, C))
    idf32 = np.eye(C, dtype=np.float32)
    ones = np.ones((1, HW), np.float32)
    maps = []
    for core in range(N_CORES):
        bi, h = core // 2, core % 2
        perm = np.concatenate([np.arange(h * MH, (h + 1) * MH),
                               np.arange((1 - h) * MH, (2 - h) * MH)])
        xin_p = np.ascontiguousarray(np.vstack([xr[bi][:, perm], ones]))
        xlat_p = np.ascontiguousarray(np.vstack([xlr[bi][:, perm], ones]))
        maps.append({"xin": xin_p, "xlat": xlat_p, "weT": weT, "be_t": be_t,
                     "idf32": idf32, "wb_all": wb_all})
    return maps


def run(inputs, trace=False, trace_cores=None):
    if "nc" not in _CACHE:
        _CACHE["nc"] = build()
    nc = _CACHE["nc"]

    names = ["high", "low", "value", "e_conv", "mid", "latter"]
    W = {n: np.asarray(inputs[f"W_{n}"], dtype=np.float32) for n in names}
    b = {n: np.asarray(inputs[f"b_{n}"], dtype=np.float32) for n in names}
    x = np.asarray(inputs["x"], dtype=np.float32)
    x_latter = np.asarray(inputs["x_latter"], dtype=np.float32)
    maps = _prep_inputs(x_latter, x, W, b)

    kw = {}
    if trace:
        kw = dict(trace=True,
                  trace_cores=trace_cores or list(range(N_CORES)))
    res = run_bass_kernel_spmd(nc, maps, core_ids=list(range(N_CORES)), **kw)

    B = x_latter.shape[0]
    out = np.empty((B, C, HW), dtype=np.float32)
    for core in range(N_CORES):
        bi, h = core // 2, core % 2
        out[bi][:, h * MH:(h + 1) * MH] = res.results[core]["outp"]
    H = int(np.sqrt(HW))
    return out.reshape(B, C, H, H), res


def kernel(**inputs):
    out, _ = run(inputs, trace=False)
    return out
